# revision 21
# baseline (speedup 1.0000x reference)
"""AffinityPropagate3 Trainium2 kernel.

Reference semantics (per batch sample, run on one NeuronCore):
    K = softmax(guided, axis=0)             # (9, H, W)
    mask = sign(sparse_depth)               # {0,1}
    x_{t+1} = mask*x0 + (1-mask) * sum_k K_k * shift_k(x_t),  16 steps,
    3x3 shifts with zero padding.

Sharding: pure data parallel, one batch sample per core (B=8, 8 cores).

On-chip layout: 120 partitions x 4 rows each (480 rows), each row padded
to 642 cols (one zero pad col per side), plus one halo row above/below
-> x buffer xa is [120, 6, 642] fp16.  All 9 stencil taps become
free-dim offset reads.  A second copy xb, shifted left by one element,
provides 4-byte-aligned views for the dw=0 taps so DVE 16-bit 2x mode
stays enabled.

Steady state per iteration, split into row-halves A (rows 0-1) and B
(rows 2-3) of every partition:
  - DVE: 5 multi-plane fp16 tensor_tensor multiplies per half (9 taps,
    grouped via overlapping strided access patterns), halo-dependent
    groups emitted last so halo DMAs from the previous iteration are
    off the critical path.
  - TensorE: identity matmuls accumulate m0 + sum_k z_k into per-half
    PSUM fp32 (512/512/256 col chunks).
  - ACT: per-half PSUM -> xa and xb writebacks (row 3 first, so the
    down-halo DMAs can fire early).
  - DMA: 4 partition-shifted SBUF->SBUF halo row refreshes.
Softmax weights (exp / denom * nomask) are computed in a DMA/exp/PE
pipeline chunked by the same row-halves, with guided loads alternating
between two DMA queues (sync + scalar) to reach full HBM bandwidth.
Iteration 1's half-A products are emitted between the two softmax
tails so the DVE FIFO overlaps them with half-B's loads.
"""

import sys

for _p in ("/root/.axon_site/_ro/trn_rl_repo", "/opt/trn_rl_repo"):
    while _p in sys.path:
        sys.path.remove(_p)
    sys.path.insert(0, _p)

import numpy as np

from concourse import bacc, mybir
from concourse import tile
import concourse.bass_utils as _bass_utils
from concourse.bass_utils import run_bass_kernel_spmd


def dedup_ldweights(nc):
    """Every matmul in this kernel uses the same stationary identity
    matrix; drop all but the first InstLdweights (PE weights persist
    across matmuls).  An Ldweights carrying sync info becomes a NoOp so
    its waits/updates still fire."""
    for f in nc.m.functions:
        for bb in f.blocks:
            out = []
            seen_key = None
            changed = False
            for ins in bb.instructions:
                if type(ins).__name__ == "InstLdweights":
                    key = str(ins.ins[0])
                    if key == seen_key:
                        si = ins.sync_info
                        if si is not None and (si.on_wait or si.on_update):
                            out.append(
                                mybir.InstNoOp(
                                    name=ins.name + "-ldwn",
                                    engine=ins.engine,
                                    sync_info=si,
                                )
                            )
                        changed = True
                        continue
                    seen_key = key
                out.append(ins)
            if changed:
                bb.instructions[:] = out


B = 8
H, W = 480, 640
P = 120          # partitions used
RPP = 4          # rows per partition
WP = W + 2       # padded row width
NJ = RPP + 2     # row slots incl. halo
FLAT = RPP * W   # 2560 free elems per partition
HFLAT = 2 * W    # 1280 free elems per half
PROP_TIME = 16
CHUNKS = [(512, 512), (1024, 256), (0, 512)]  # psum col chunks per half (row-3 banks first)

FP32 = mybir.dt.float32
FP16 = mybir.dt.float16

# 3x3 tap order matching torch unfold channel order: k = ki*3 + kj,
# patches[k][h, w] = x[h + ki - 1, w + kj - 1].
# wk plane order (grouped by op):
#   planes 0,1: (dh=0, dw=-1), (0,+1)      unfold k = 3, 5
#   plane  2  : (0, 0)                      k = 4
#   planes 3,4: (-1,-1), (-1,+1)            k = 0, 2
#   planes 5,6: (+1,-1), (+1,+1)            k = 6, 8
#   planes 7,8: (-1,0), (+1,0)              k = 1, 7
PLANE_OF_K = {3: 0, 5: 1, 4: 2, 0: 3, 2: 4, 6: 5, 8: 6, 1: 7, 7: 8}


def _rows_view(dram_ap):
    """DRAM [H, W] -> [P, RPP, W]."""
    return dram_ap.rearrange("(p r) w -> p r w", p=P)


def _rows2_view(dram_ap, h):
    """DRAM [H, W] -> [P, 2, W] covering row pair h of each partition."""
    return dram_ap.rearrange("(p r) w -> p r w", p=P)[:, 2 * h : 2 * h + 2, :]


def _xview(xflat, offset, dims):
    """Custom (possibly overlapping) free-dim AP into a flat [P, NJ*WP]
    buffer view."""
    c = xflat.copy()
    c.ap = c.ap[:1] + dims
    c.offset = offset
    return c


def _bcast(apv, n):
    """[P, 2, W] -> [P, n, 2, W] with 0-stride plane dim."""
    c = apv.copy()
    c.ap = c.ap[:1] + [[0, n]] + list(c.ap[1:])
    return c


def build_program(compile_=True):
    nc = bacc.Bacc("TRN2", target_bir_lowering=False, debug=False, num_devices=B)

    guided_d = nc.dram_tensor("guided", [9, H, W], FP32, kind="ExternalInput")
    x_d = nc.dram_tensor("x", [H, W], FP32, kind="ExternalInput")
    sparse_d = nc.dram_tensor("sparse_depth", [H, W], FP32, kind="ExternalInput")
    out_d = nc.dram_tensor("out", [H, W], FP32, kind="ExternalOutput")

    ident_d = nc.inline_tensor(np.eye(P, dtype=np.float16), name="ident_const")

    with tile.TileContext(nc) as tc:
        with (
            tc.tile_pool(name="persist", bufs=1) as persist,
            tc.tile_pool(name="work32", bufs=6) as work32,
            tc.tile_pool(name="psum", bufs=1, space="PSUM") as psump,
        ):
            # ---- persistent buffers ----
            xa = persist.tile([P, NJ, WP], FP16, tag="xa")
            xb = persist.tile([P, NJ, WP], FP16, tag="xb")
            wk = persist.tile([P, 9, RPP, W], FP16, tag="wk")
            za = persist.tile([P, 9, 2, W], FP16, tag="za")
            zb = persist.tile([P, 9, 2, W], FP16, tag="zb")
            m0 = persist.tile([P, RPP, W], FP16, tag="m0")
            nomask = persist.tile([P, RPP, W], FP16, tag="nomask")
            rf = [
                persist.tile([P, 2, W], FP16, tag=f"rf{h}", name=f"rf{h}")
                for h in range(2)
            ]
            ident = persist.tile([P, P], FP16, tag="ident")
            r32 = persist.tile([P, HFLAT], FP32, tag="r32")
            stag = persist.tile([P, RPP, W], FP32, tag="stag")

            psum = [
                psump.tile([P, HFLAT], FP32, tag=f"psum{h}", name=f"psum{h}")
                for h in range(2)
            ]

            nc.vector.memset(xa[:], 0.0)
            nc.vector.memset(xb[:], 0.0)
            nc.sync.dma_start(out=ident[:], in_=ident_d[:])

            # ---- x load (cast fp32->fp16 via SWDGE) ----
            xd = _rows_view(x_d[:])
            nc.gpsimd.dma_start(out=xa[:, 1 : 1 + RPP, 1 : 1 + W], in_=xd)
            nc.gpsimd.dma_start(
                out=xa[1:P, 0:1, 1 : 1 + W], in_=xd[0 : P - 1, 3:4, :]
            )
            nc.gpsimd.dma_start(
                out=xa[0 : P - 1, 5:6, 1 : 1 + W], in_=xd[1:P, 0:1, :]
            )
            # xb = xa shifted left one element (flat)
            nflat = NJ * WP
            xaf = xa.rearrange("p a b -> p (a b)")
            xbf = xb.rearrange("p a b -> p (a b)")
            nc.vector.tensor_copy(
                out=xbf[:, 0 : nflat - 1], in_=xaf[:, 1:nflat]
            )

            # ---- masks ----
            sp = persist.tile([P, RPP, W], FP32, tag="sp32")
            nc.gpsimd.dma_start(out=sp[:], in_=_rows_view(sparse_d[:]))
            nc.vector.tensor_scalar(
                out=nomask[:], in0=sp[:], scalar1=0.0,
                scalar2=None, op0=mybir.AluOpType.is_equal,
            )
            xv = xa[:, 1 : 1 + RPP, 1 : 1 + W]
            nc.vector.tensor_tensor(
                out=m0[:], in0=nomask[:], in1=xv, op=mybir.AluOpType.mult
            )
            nc.vector.tensor_tensor(
                out=m0[:], in0=xv, in1=m0[:], op=mybir.AluOpType.subtract
            )

            # ---- softmax loads: half-row loads interleaved with exp
            # on ACT + per-half denominator accumulation on TensorE.
            # (HBM read bw for this load is ~180 GB/s regardless of queue
            # arrangement — measured; half-granularity lets half 0's
            # softmax tail and iteration 1 overlap the half-1 loads.) ----
            def emit_softmax_loads(h):
                for k in range(9):
                    pl = PLANE_OF_K[k]
                    g32 = work32.tile([P, 2, W], FP32, tag="g32")
                    gv = _rows2_view(guided_d[k], h)
                    for qi in range(2):
                        q = nc.sync if ((k + qi) % 2 == 0) else nc.scalar
                        q.dma_start(
                            out=g32[:, qi : qi + 1, :],
                            in_=gv[:, qi : qi + 1, :],
                        )
                    nc.scalar.activation(
                        out=wk[:, pl, 2 * h : 2 * h + 2, :], in_=g32[:],
                        func=mybir.ActivationFunctionType.Exp,
                    )
                    wflat = wk[:, pl, 2 * h : 2 * h + 2, :].rearrange(
                        "p a b -> p (a b)"
                    )
                    for o, n in CHUNKS:
                        nc.tensor.matmul(
                            out=psum[h][:, o : o + n],
                            lhsT=ident[:],
                            rhs=wflat[:, o : o + n],
                            start=(k == 0),
                            stop=(k == 8),
                        )

            def emit_softmax_tail(h, planes):
                # reciprocal of denominator, nomask fold (first call),
                # then in-place normalization of the given wk planes
                if planes[0] == 0:
                    nc.vector.reciprocal_approx_fast(
                        out=r32[:], in_=psum[h][:]
                    )
                    nc.vector.tensor_tensor(
                        out=rf[h][:],
                        in0=r32.rearrange("p (a b) -> p a b", a=2)[:],
                        in1=nomask[:, 2 * h : 2 * h + 2, :],
                        op=mybir.AluOpType.mult,
                    )
                wall = wk[:, planes[0] : planes[1], 2 * h : 2 * h + 2, :]
                nc.vector.tensor_tensor(
                    out=wall, in0=wall,
                    in1=_bcast(rf[h][:], planes[1] - planes[0]),
                    op=mybir.AluOpType.mult,
                )

            # ---- iteration building blocks ----
            z = [za, zb]

            def emit_mms(h, planes, stop=False):
                for i, pl in enumerate(planes):
                    zf = z[h][:, pl].rearrange("p a b -> p (a b)")
                    for o, n in CHUNKS:
                        nc.tensor.matmul(
                            out=psum[h][:, o : o + n],
                            lhsT=ident[:],
                            rhs=zf[:, o : o + n],
                            start=False,
                            stop=stop and (i == len(planes) - 1),
                        )

            def emit_m0(h):
                mf = m0[:, 2 * h : 2 * h + 2, :].rearrange("p a b -> p (a b)")
                for o, n in CHUNKS:
                    nc.tensor.matmul(
                        out=psum[h][:, o : o + n],
                        lhsT=ident[:],
                        rhs=mf[:, o : o + n],
                        start=True,
                        stop=False,
                    )

            def op_pair(h, p0, base_slot):
                """planes p0,p0+1 = (dw=-1, dw=+1) from xa at row slot
                base_slot; view [j:2@2][r:2@WP][c:W@1]."""
                nc.vector.tensor_tensor(
                    out=z[h][:, p0 : p0 + 2],
                    in0=wk[:, p0 : p0 + 2, 2 * h : 2 * h + 2, :],
                    in1=_xview(xaf, base_slot * WP, [[2, 2], [WP, 2], [1, W]]),
                    op=mybir.AluOpType.mult,
                )

            def op_xb_mid(h):
                # plane 2: (dh=0, dw=0) from xb
                nc.vector.tensor_tensor(
                    out=z[h][:, 2],
                    in0=wk[:, 2, 2 * h : 2 * h + 2, :],
                    in1=xb[:, 1 + 2 * h : 3 + 2 * h, 0:W],
                    op=mybir.AluOpType.mult,
                )

            def op_xb_vert(h):
                # planes 7,8: (dh=-1, dw=0), (dh=+1, dw=0) from xb
                nc.vector.tensor_tensor(
                    out=z[h][:, 7:9],
                    in0=wk[:, 7:9, 2 * h : 2 * h + 2, :],
                    in1=_xview(
                        xbf, (2 * h) * WP, [[2 * WP, 2], [WP, 2], [1, W]]
                    ),
                    op=mybir.AluOpType.mult,
                )

            def emit_products(h, skip_head=False):
                """All 9 tap products + m0/psum accumulation for half h,
                halo-dependent ops last."""
                if not skip_head:
                    emit_m0(h)
                if h == 0:
                    if not skip_head:
                        op_pair(0, 0, 1)    # dh=0, dw=+-1
                        emit_mms(0, (0, 1))
                        op_xb_mid(0)        # dh=0, dw=0
                        emit_mms(0, (2,))
                    op_pair(0, 5, 2)        # dh=+1, dw=+-1
                    emit_mms(0, (5, 6))
                    op_pair(0, 3, 0)        # dh=-1 (slot0 halo)
                    emit_mms(0, (3, 4))
                    op_xb_vert(0)           # dh=+-1, dw=0 (xb slot0 halo)
                    emit_mms(0, (7, 8), stop=True)
                else:
                    op_pair(1, 0, 3)        # dh=0, dw=+-1
                    emit_mms(1, (0, 1))
                    op_xb_mid(1)            # dh=0, dw=0
                    emit_mms(1, (2,))
                    op_pair(1, 3, 2)        # dh=-1
                    emit_mms(1, (3, 4))
                    op_pair(1, 5, 4)        # dh=+1 (slot5 halo)
                    emit_mms(1, (5, 6))
                    op_xb_vert(1)           # dh=+-1, dw=0 (xb slot5 halo)
                    emit_mms(1, (7, 8), stop=True)

            def emit_evac(last):
                pa = psum[0].rearrange("p (a b) -> p a b", a=2)
                pb = psum[1].rearrange("p (a b) -> p a b", a=2)
                if not last:
                    # evac half A (rows 0,1), then up-halos (row 0)
                    nc.scalar.copy(out=xa[:, 1:3, 1 : 1 + W], in_=pa[:])
                    nc.sync.dma_start(
                        out=xa[0 : P - 1, 5:6, 1 : 1 + W],
                        in_=xa[1:P, 1:2, 1 : 1 + W],
                    )
                    nc.sync.dma_start(
                        out=xb[0 : P - 1, 5:6, 0:W], in_=xa[1:P, 1:2, 1 : 1 + W]
                    )
                    nc.scalar.copy(out=xb[:, 1:3, 0:W], in_=pa[:])
                    # evac half B: row 3 first, fire down-halos
                    nc.scalar.copy(out=xa[:, 4:5, 1 : 1 + W], in_=pb[:, 1:2])
                    nc.sync.dma_start(
                        out=xa[1:P, 0:1, 1 : 1 + W],
                        in_=xa[0 : P - 1, 4:5, 1 : 1 + W],
                    )
                    nc.sync.dma_start(
                        out=xb[1:P, 0:1, 0:W], in_=xa[0 : P - 1, 4:5, 1 : 1 + W]
                    )
                    nc.scalar.copy(out=xb[:, 3:4, 0:W], in_=pb[:, 0:1])
                    nc.scalar.copy(out=xa[:, 3:4, 1 : 1 + W], in_=pb[:, 0:1])
                    nc.scalar.copy(out=xb[:, 4:5, 0:W], in_=pb[:, 1:2])
                else:
                    nc.scalar.copy(out=stag[:, 0:2], in_=pa[:])
                    nc.sync.dma_start(
                        out=_rows_view(out_d[:])[:, 0:2, :], in_=stag[:, 0:2]
                    )
                    nc.scalar.copy(out=stag[:, 2:3], in_=pb[:, 0:1])
                    nc.scalar.dma_start(
                        out=_rows_view(out_d[:])[:, 2:3, :], in_=stag[:, 2:3]
                    )
                    nc.scalar.copy(out=stag[:, 3:4], in_=pb[:, 1:2])
                    nc.sync.dma_start(
                        out=_rows_view(out_d[:])[:, 3:4, :], in_=stag[:, 3:4]
                    )

            # ---- preproc + iteration 1, interleaved for DVE overlap:
            # half-0 softmax tail and iter-1 half A run under half-1
            # loads; normalization split three ways so each iter-1
            # product op starts as soon as its planes are scaled ----
            emit_softmax_loads(0)
            emit_softmax_loads(1)
            emit_softmax_tail(0, (0, 2))
            emit_m0(0)
            op_pair(0, 0, 1)
            emit_mms(0, (0, 1))
            emit_softmax_tail(0, (2, 5))
            op_xb_mid(0)
            emit_mms(0, (2,))
            emit_softmax_tail(0, (5, 9))
            op_pair(0, 5, 2)
            emit_mms(0, (5, 6))
            op_pair(0, 3, 0)
            emit_mms(0, (3, 4))
            op_xb_vert(0)
            emit_mms(0, (7, 8), stop=True)
            # iter-2 half-A head: depends only on iter-1's half-A evac,
            # emitted here so the DVE FIFO overlaps it with half-1 loads
            emit_softmax_tail(1, (0, 2))
            emit_m0(1)
            op_pair(1, 0, 3)
            emit_mms(1, (0, 1))
            emit_softmax_tail(1, (2, 5))
            op_xb_mid(1)
            emit_mms(1, (2,))
            emit_softmax_tail(1, (5, 9))
            op_pair(1, 3, 2)
            emit_mms(1, (3, 4))
            op_pair(1, 5, 4)
            emit_mms(1, (5, 6))
            op_xb_vert(1)
            emit_mms(1, (7, 8), stop=True)
            emit_evac(last=False)

            # hoisted head of iteration 2 (half A: m0 + first two ops)
            emit_m0(0)
            op_pair(0, 0, 1)
            emit_mms(0, (0, 1))
            op_xb_mid(0)
            emit_mms(0, (2,))

            # ---- iterations 2..16 ----
            for t in range(1, PROP_TIME):
                emit_products(0, skip_head=(t == 1))
                emit_products(1)
                emit_evac(last=(t == PROP_TIME - 1))

    if compile_:
        dedup_ldweights(nc)
        nc.compile()
    return nc


_CACHED_NC = None


def _get_nc():
    global _CACHED_NC
    if _CACHED_NC is None:
        _CACHED_NC = build_program()
    return _CACHED_NC


def kernel(guided, x, sparse_depth, _trace=False, _trace_kwargs=None):
    guided = np.ascontiguousarray(guided, dtype=np.float32)
    x = np.ascontiguousarray(x, dtype=np.float32)
    sparse_depth = np.ascontiguousarray(sparse_depth, dtype=np.float32)
    assert guided.shape == (B, 9, H, W)

    nc = _get_nc()
    in_maps = [
        {
            "guided": guided[b],
            "x": x[b, 0],
            "sparse_depth": sparse_depth[b, 0],
        }
        for b in range(B)
    ]
    res = run_bass_kernel_spmd(
        nc, in_maps, list(range(B)), trace=_trace, **(_trace_kwargs or {})
    )
    out = np.stack([res.results[b]["out"] for b in range(B)])[:, None]
    if _trace:
        return out.astype(np.float32), res
    return out.astype(np.float32)
